# revision 27
# baseline (speedup 1.0000x reference)
"""Inner-policy-sharded Trainium2 kernel for DecoupledDynamicsModel (MoE),
fp8-DoubleRow edition.

Model: B=8192 rows; each row selects one of P=8 outer policies via
policy_indices; the selected policy runs 8 inner MLPs (72 -> 512 -> 512 -> 64)
on (latent chunk, action); the 8 inner outputs concatenate to 512 dims.

Sharding: by INNER policy (as the previous fp32r kernel): core i computes
inner MLP i for every row, rows sorted by outer policy so weights are
stationary per contiguous group.

Compute strategy (vs the 24 cyc/token fp32r baseline -> 16 cyc/token):
 - Layer 1 (K=72): fp8 e4m3 DoubleRow matmul with an error-compensated
   3-term expansion packed into one K=216 contraction:
       W1q.x_hi + W1q.x_lo + r1.x_hi   (x = x_hi + x_lo exact fp8 pair,
                                        W1 = W1q + r1 fp8 pair)
   packed [108, 2, *] -> ONE DR matmul per m-chunk = 0.5 cyc/row -> 2T.
 - Layer 2 (K=512): same 3-term scheme, 6 DR matmuls per m-chunk -> 12T.
   The moving pair (h_hi, h_lo) is produced on-chip: ACT evicts
   h = s_h*relu(psum*a + b) to bf16, a software-DGE SBUF->SBUF casting DMA
   makes h_hi = fp8(h), and DVE/Pool subtract h - h_hi -> h_lo (fp8).
   The hi+lo pair reconstructs h exactly up to fp8(h_lo) rounding, so the
   dropped r2.h_lo cross term is the only systematic error (~0.4%).
 - Layer 3: flipped orientation bf16: stationary = h2 [128h, 128tok]
   chunks, moving = W3 [128, 64] -> out [tok, 64] in PSUM, 2T.
Measured end-to-end emulation rel_err ~3.6e-3 (gate 2e-2).

All scales are powers of two folded into ACT scale/bias; the final
1/(s_h*sw2) descale and +b3 happen on the host after the gather.
"""

import sys

sys.path.insert(0, "/opt/trn_rl_repo")

import numpy as np
import ml_dtypes

import concourse.bass as bass
from concourse import bacc
import concourse.mybir as mybir
import concourse.tile as tile
from concourse.bass_utils import run_bass_kernel_spmd

P = 8          # outer policies == n_cores == inner MLPs per policy
Z = 64         # per-policy latent dim
D = P * Z      # 512
A = 8          # action dim
IN = Z + A     # 72
H = 512        # hidden dim
NCORES = 8

SX = 16.0      # x fp8 scale
SW1 = 2.0      # W1 fp8 scale (SX*SW1 == SH so layer-1 eviction has scale 1)
SH = 32.0      # h fp8/bf16 scale
SW2 = 2048.0   # W2 fp8 scale
A1 = SH / (SX * SW1)     # ACT scale for layer-1 eviction (2^-8)
SIG2 = SH * SW2          # scale of layer-2/3 psum; undone on host

W8C = 5120     # fp8 weight tile columns per group: w1 1024 | wq2a/b | r2a/b
W16C = 256     # bf16 tile: W3 [128, 4*64]

F8 = mybir.dt.float8e4
BF = mybir.dt.bfloat16
F32 = mybir.dt.float32
RELU = mybir.ActivationFunctionType.Relu
COPY = mybir.ActivationFunctionType.Copy
DR = mybir.MatmulPerfMode.DoubleRow
ADD = mybir.AluOpType.add
MAX = mybir.AluOpType.max
SUB = mybir.AluOpType.subtract
NF8 = ml_dtypes.float8_e4m3
NBF = ml_dtypes.bfloat16

TRACE = False
REPEAT = 1
LAST_RESULT = None


def _q8(v):
    """Round to fp8 e4m3 (framework dtype), back to f32."""
    return v.astype(NF8).astype(np.float32)


def _group_tiles(counts):
    """Token tiles of <=512 inside each outer-policy group (counts padded
    to multiples of 4). Group 0 leads with small tiles so the software
    pipeline (depth 4) fills while DMAs are still streaming."""
    tiles = []
    off = 0
    for g, n in enumerate(counts):
        r = n
        lead = [128, 128, 256] if g == 0 else []
        for t in lead:
            if r >= t + 128:
                tiles.append((g, off, t))
                off += t
                r -= t
        while r > 0:
            t = 512 if r >= 512 else r
            tiles.append((g, off, t))
            off += t
            r -= t
    return tiles


def _subtile_plan(tiles):
    """Global subtile index space for the token-major y output: per tile a
    (j0, widths) entry; widths are <=128 chunks of the tile."""
    plan = []
    j = 0
    for (_g, _t0, tw) in tiles:
        widths = []
        r = tw
        while r > 0:
            w = 128 if r >= 128 else r
            widths.append(w)
            r -= w
        plan.append((j, widths))
        j += len(widths)
    return plan, j


def _build_program(counts, B, repeat=1):
    tiles = _group_tiles(counts)
    plan, nsub = _subtile_plan(tiles)
    NT = len(tiles)

    nc = bacc.Bacc()
    xd = nc.declare_dram_parameter("x8", [108, 2, B], F8, isOutput=False)
    w8d = nc.declare_dram_parameter("w8", [P, 128, W8C], F8, isOutput=False)
    w16d = nc.declare_dram_parameter("w16", [128, P * 4, 64], BF, isOutput=False)
    w32d = nc.declare_dram_parameter("w32", [128, P * 8], F32, isOutput=False)
    yd = nc.declare_dram_parameter("y", [128, nsub, 64], F32, isOutput=True)

    with tile.TileContext(nc) as tc:
        with (
            tc.tile_pool(name="xs", bufs=1) as xpool,
            tc.tile_pool(name="w", bufs=P) as wpool,
            tc.tile_pool(name="b", bufs=1) as bpool,
            tc.tile_pool(name="h", bufs=5) as hpool,
            tc.tile_pool(name="ys", bufs=2) as ypool,
            tc.tile_pool(name="ps1", bufs=7, space="PSUM") as pp1,
            tc.tile_pool(name="ps3", bufs=1, space="PSUM") as pp3,
        ):
            for _rep in range(repeat):
                xt = xpool.tile([108, 2, B], F8, tag="x")
                bia = bpool.tile([128, P * 8], F32, tag="bias")
                w3a = bpool.tile([128, P * 4, 64], BF, tag="w3a")
                # one packed fp8 weight tile per group; regions are 3D views
                wg8 = []
                for _g in range(P):
                    wg8.append(
                        wpool.tile([128, 10, 512], F8, tag="wg8", name=f"wg8_{_g}")
                    )

                def w1s(g):
                    return wg8[g][0:108, 0:2, :]

                def wq2(g, half):
                    return wg8[g][:, 2 + 2 * half : 4 + 2 * half, :]

                def r2(g, half):
                    return wg8[g][:, 6 + 2 * half : 8 + 2 * half, :]

                # ---- just-in-time DMA chunk schedule ----
                # weight chunks sized <=1us so pair-cast transfers never
                # queue behind bulk weights on the shared DMA device
                first_slot = {}
                for k, (g, _t0, _tw) in enumerate(tiles):
                    first_slot.setdefault(g, k)

                def wchunk(g, d0, d1):
                    return lambda: nc.sync.dma_start(
                        wg8[g][:, d0:d1, :], w8d[g, :, d0 * 512 : d1 * 512]
                    )

                dma_sched = {}

                def sched(slot, thunk):
                    dma_sched.setdefault(max(slot, 0), []).append(thunk)

                for g in range(P):
                    fs = first_slot[g]
                    if g > 0:
                        sched(fs - 4, wchunk(g, 0, 2))
                    sched(fs, wchunk(g, 2, 6))
                    sched(fs + 1, wchunk(g, 6, 10))
                sched(3, lambda: nc.sync.dma_start(
                    w3a[:, 0:16, :], w16d[:, 0:16, :]))
                sched(5, lambda: nc.sync.dma_start(
                    w3a[:, 16:32, :], w16d[:, 16:32, :]))
                # x spans: cover through tile s+2 at slot s
                xcov = [0]

                def x_to(tok):
                    if tok > xcov[0]:
                        nc.sync.dma_start(
                            xt[:, :, xcov[0] : tok], xd[:, :, xcov[0] : tok]
                        )
                        xcov[0] = tok

                # prologue DMAs
                nc.sync.dma_start(bia[:, :], w32d[:, :])
                nc.sync.dma_start(wg8[0][:, 0:2, :], w8d[0, :, 0:1024])
                x_to(tiles[2][1] + tiles[2][2] if NT > 2 else B)

                # ---- software-pipelined tile loop ----
                # pair-batched h buffers: tiles 2p, 2p+1 share [128, 4, 1024]
                # packed contiguously (tile 2p+1 at column tiles[2p].tw)
                hbs, hhs, hls, h2s = {}, {}, {}, {}
                poff = {}
                for k, (_g, _t0, tw) in enumerate(tiles):
                    poff[k] = 0 if k % 2 == 0 else tiles[k - 1][2]

                # greedy engine placement: every psum eviction can run on
                # ACT (activation) or DVE (tensor_scalar); assign each op to
                # whichever projected engine load is lower. Subs split
                # DVE/Pool the same way.
                load = {"A": 0.0, "D": 0.0, "P": 0.0}

                def place_evict(dst, src, bias_ap, cols):
                    ca = cols * 0.833 + 200
                    cd = cols * 1.0417 + 195
                    if load["A"] + ca <= load["D"] + cd:
                        load["A"] += ca
                        nc.scalar.activation(dst, src, RELU, bias=bias_ap,
                                             scale=1.0)
                    else:
                        load["D"] += cd
                        nc.vector.tensor_scalar(dst, src, bias_ap, 0.0,
                                                ADD, MAX)

                def place_copy(dst, src, cols):
                    ca = cols * 0.833 + 200
                    cd = cols * 1.0417 + 195
                    if load["A"] + ca <= load["D"] + cd:
                        load["A"] += ca
                        nc.scalar.activation(dst, src, COPY)
                    else:
                        load["D"] += cd
                        nc.vector.tensor_copy(dst, src)

                def place_sub(h_lo, h_bf, h_hi, r0, r1, off, tw):
                    cols = (r1 - r0) * tw
                    cd = cols * 1.0417 + 195
                    cp = cols * 1.984 + 60
                    a = (h_lo[:, r0:r1, off : off + tw],
                         h_bf[:, r0:r1, off : off + tw],
                         h_hi[:, r0:r1, off : off + tw], SUB)
                    if load["D"] + cd <= load["P"] + cp:
                        load["D"] += cd
                        nc.vector.tensor_tensor(*a)
                    else:
                        load["P"] += cp
                        nc.gpsimd.tensor_tensor(*a)

                def stage1(k, ms):
                    g, t0, tw = tiles[k]
                    p = k // 2
                    if k % 2 == 0 and ms[0] == 0:
                        hbs[p] = hpool.tile([128, 4, 1024], BF, tag="hb", name=f"hb_{p}")
                        hhs[p] = hpool.tile([128, 4, 1024], F8, tag="hh", name=f"hh_{p}")
                        hls[p] = hpool.tile([128, 4, 1024], F8, tag="hl", name=f"hl_{p}")
                    h_bf = hbs[p]
                    off = poff[k]
                    for m in ms:
                        ps = pp1.tile([128, 512], F32, tag="p1")
                        nc.tensor.matmul(
                            ps[:, :tw],
                            w1s(g)[:, :, m * 128 : (m + 1) * 128],
                            xt[:, :, t0 : t0 + tw],
                            start=True,
                            stop=True,
                            perf_mode=DR,
                        )
                        place_evict(
                            h_bf[:, m, off : off + tw], ps[:, :tw],
                            bia[:, g * 8 + m : g * 8 + m + 1], tw,
                        )

                def cast_subs(p, klast):
                    # one width-exact casting DMA for the pair, then subs;
                    # kpair0 subs pinned to DVE (their h_lo mms come first)
                    h_bf, h_hi, h_lo = hbs[p], hhs[p], hls[p]
                    W = poff[klast] + tiles[klast][2]
                    nc.gpsimd.dma_start(h_hi[:, :, :W], h_bf[:, :, :W])
                    load["P"] += 994 + W / 8
                    for k in range(2 * p, klast + 1):
                        _g, _t0, tw = tiles[k]
                        off = poff[k]
                        load["D"] += 2 * tw * 1.0417 + 195
                        nc.vector.tensor_tensor(
                            h_lo[:, 0:2, off : off + tw],
                            h_bf[:, 0:2, off : off + tw],
                            h_hi[:, 0:2, off : off + tw], SUB,
                        )
                        place_sub(h_lo, h_bf, h_hi, 2, 3, off, tw)
                        place_sub(h_lo, h_bf, h_hi, 3, 4, off, tw)

                def stage2(k):
                    g, t0, tw = tiles[k]
                    p, half = k // 2, k % 2
                    h_hi, h_lo = hhs[p], hls[p]
                    o = poff[k]
                    h2 = hpool.tile([128, 4, 512], BF, tag="h2")
                    h2s[k] = h2
                    # all 16 h_hi mms first (the cast unblocks them), then
                    # the 8 h_lo mms - the subs get ~1.5us of extra slack
                    pss = []
                    for m in range(4):
                        msl = slice(m * 128, (m + 1) * 128)
                        ps = pp1.tile([128, 512], F32, tag="p1")
                        pss.append(ps)
                        for j, (w, mv) in enumerate([
                            (wq2(g, 0), h_hi[:, 0:2, o : o + tw]),
                            (wq2(g, 1), h_hi[:, 2:4, o : o + tw]),
                            (r2(g, 0), h_hi[:, 0:2, o : o + tw]),
                            (r2(g, 1), h_hi[:, 2:4, o : o + tw]),
                        ]):
                            nc.tensor.matmul(
                                pss[m][:, :tw], w[:, :, msl], mv,
                                start=(j == 0), stop=False,
                                perf_mode=DR,
                            )
                    for m in range(4):
                        msl = slice(m * 128, (m + 1) * 128)
                        nc.tensor.matmul(
                            pss[m][:, :tw], wq2(g, 0)[:, :, msl],
                            h_lo[:, 0:2, o : o + tw],
                            start=False, stop=False, perf_mode=DR,
                        )
                    for m in range(4):
                        msl = slice(m * 128, (m + 1) * 128)
                        nc.tensor.matmul(
                            pss[m][:, :tw], wq2(g, 1)[:, :, msl],
                            h_lo[:, 2:4, o : o + tw],
                            start=False, stop=True, perf_mode=DR,
                        )
                        place_evict(
                            h2[:, m, :tw], pss[m][:, :tw],
                            bia[:, g * 8 + 4 + m : g * 8 + 4 + m + 1], tw,
                        )
                    if half == 1 or k == NT - 1:
                        hbs.pop(p), hhs.pop(p), hls.pop(p)

                def stage3(k):
                    g, t0, tw = tiles[k]
                    j0, widths = plan[k]
                    h2 = h2s.pop(k)
                    ps3 = pp3.tile([128, 4, 64], F32, tag="p3")
                    for s, w in enumerate(widths):
                        for kc in range(4):
                            nc.tensor.matmul(
                                ps3[0:w, s, :],
                                h2[:, kc, s * 128 : s * 128 + w],
                                w3a[:, g * 4 + kc, :],
                                start=(kc == 0),
                                stop=(kc == 3),
                            )
                    y = ypool.tile([128, 4, 64], F32, tag="y")
                    ns = len(widths)
                    nfull = ns if widths[-1] == 128 else ns - 1
                    if nfull:
                        place_copy(y[:, 0:nfull, :], ps3[:, 0:nfull, :],
                                   nfull * 64)
                        nc.sync.dma_start(
                            yd[:, j0 : j0 + nfull, :], y[:, 0:nfull, :]
                        )
                    if nfull < ns:
                        wl = widths[-1]
                        place_copy(
                            y[0:wl, nfull : nfull + 1, :],
                            ps3[0:wl, nfull : nfull + 1, :], 64,
                        )
                        nc.sync.dma_start(
                            yd[0:wl, j0 + nfull : j0 + nfull + 1, :],
                            y[0:wl, nfull : nfull + 1, :],
                        )

                for s in range(NT + 8):
                    for thunk in dma_sched.get(s, []):
                        thunk()
                    if s % 2 == 0 and s + 3 < NT:
                        x_to(tiles[s + 3][1] + tiles[s + 3][2])
                    elif s + 3 >= NT and xcov[0] < B:
                        x_to(B)
                    if s < NT:
                        stage1(s, (0, 1, 2))
                    if 0 <= s - 6 < NT:
                        stage2(s - 6)
                    if 0 <= s - 7 < NT:
                        stage3(s - 7)
                    if s < NT:
                        stage1(s, (3,))
                        if s % 2 == 1 or s == NT - 1:
                            cast_subs(s // 2, s)

    nc.finalize()
    return nc


def _pack_inputs(latents, actions, order, counts, Bp, W1, b1, W2, b2, W3, b3):
    """Per-core inputs. Core i: fp8 pair-packed x and per-(g, inner-i)
    quantized weights."""
    lat_s = latents[order]
    act_s = actions[order]
    # sorted+padded x, zero in pad columns
    spans = []
    po = ro = 0
    for n, pn in zip(counts, [c for c in counts]):
        pass
    in_maps = []
    # padded group layout
    offs = []
    po = ro = 0
    for n in counts:
        offs.append((po, ro, n))
        pn = -(-n // 4) * 4
        po += pn
        ro += n

    for i in range(NCORES):
        xf = np.zeros((IN, Bp), dtype=np.float32)
        for po_, ro_, n in offs:
            xf[:Z, po_ : po_ + n] = lat_s[ro_ : ro_ + n, i * Z : (i + 1) * Z].T
            xf[Z:, po_ : po_ + n] = act_s[ro_ : ro_ + n].T
        xs = xf * SX
        x_hi = _q8(xs)
        x_lo = _q8(xs - x_hi)
        flat = np.zeros((216, Bp), dtype=np.float32)
        flat[0:72] = x_hi
        flat[72:144] = x_lo
        flat[144:216] = x_hi
        x8 = flat.reshape(2, 108, Bp).transpose(1, 0, 2).astype(NF8)

        w8 = np.zeros((P, 128, W8C), dtype=NF8)
        w16 = np.zeros((128, P * 4, 64), dtype=NBF)
        w32 = np.zeros((128, P * 8), dtype=np.float32)
        for g in range(P):
            w1sc = W1[g, i] * SW1                    # [72, 512]
            w1q = _q8(w1sc)
            r1 = _q8(w1sc - w1q)
            flatw = np.concatenate([w1q, w1q, r1], 0)  # [216, 512]
            # [108, 2, 512] with flat row f = j*108+p -> cols j*512+c
            w8[g, 0:108, 0:1024] = (
                flatw.reshape(2, 108, 512).transpose(1, 0, 2).reshape(108, 1024)
            ).astype(NF8)

            w2sc = W2[g, i] * SW2                    # [512, 512]
            w2q = _q8(w2sc)
            r2 = _q8(w2sc - w2q)
            for src, base in ((w2q, 1024), (r2, 3072)):
                kt = src.reshape(4, 128, 512)        # [ktile, p, col]
                w8[g, :, base : base + 1024] = (
                    kt[0:2].transpose(1, 0, 2).reshape(128, 1024).astype(NF8)
                )
                w8[g, :, base + 1024 : base + 2048] = (
                    kt[2:4].transpose(1, 0, 2).reshape(128, 1024).astype(NF8)
                )

            w16[:, g * 4 : g * 4 + 4, :] = (
                W3[g, i].reshape(4, 128, Z).transpose(1, 0, 2)
            ).astype(NBF)
            w32[:, g * 8 : g * 8 + 4] = (SH * b1[g, i]).reshape(4, 128).T
            w32[:, g * 8 + 4 : g * 8 + 8] = (SIG2 * b2[g, i]).reshape(4, 128).T

        in_maps.append({"x8": x8, "w8": w8, "w16": w16, "w32": w32})
    return in_maps


def _prepare(latents, actions, policy_indices, W1, b1, W2, b2, W3, b3):
    latents = np.asarray(latents, dtype=np.float32)
    actions = np.asarray(actions, dtype=np.float32)
    idx = np.asarray(policy_indices).astype(np.int64)
    W1 = np.ascontiguousarray(np.asarray(W1, dtype=np.float32))
    W2 = np.ascontiguousarray(np.asarray(W2, dtype=np.float32))
    W3 = np.ascontiguousarray(np.asarray(W3, dtype=np.float32))
    b1 = np.asarray(b1, dtype=np.float32)
    b2 = np.asarray(b2, dtype=np.float32)
    b3 = np.asarray(b3, dtype=np.float32)

    order = np.argsort(idx, kind="stable")
    counts = np.bincount(idx, minlength=P).tolist()
    pcounts = [-(-n // 4) * 4 for n in counts]
    Bp = sum(pcounts)

    in_maps = _pack_inputs(
        latents, actions, order, counts, Bp, W1, b1, W2, b2, W3, b3
    )
    nc = _build_program(pcounts, Bp, repeat=REPEAT)
    return nc, in_maps, order, counts, pcounts


def _scatter_out(results, order, counts, pcounts, B, idx_sorted_groups, b3):
    tiles = _group_tiles(pcounts)
    plan, nsub = _subtile_plan(tiles)
    Bp = sum(pcounts)
    keep = np.zeros(Bp, dtype=bool)
    po = 0
    for n, pn in zip(counts, pcounts):
        keep[po : po + n] = True
        po += pn
    # group id per padded row (for b3)
    grow = np.zeros(Bp, dtype=np.int64)
    po = 0
    for g, pn in enumerate(pcounts):
        grow[po : po + pn] = g
        po += pn

    out = np.empty((B, D), dtype=np.float32)
    for i in range(NCORES):
        yO = results[i]["y"]                      # [128, nsub, 64]
        ys = np.empty((Bp, Z), dtype=np.float32)
        for (g, t0, tw), (j0, widths) in zip(tiles, plan):
            o = t0
            for s, w in enumerate(widths):
                ys[o : o + w] = yO[0:w, j0 + s, :]
                o += w
        ys = ys / SIG2 + b3[grow, i]
        out[order, i * Z : (i + 1) * Z] = ys[keep]
    return out


def kernel(latents, actions, policy_indices, W1, b1, W2, b2, W3, b3):
    global LAST_RESULT
    nc, in_maps, order, counts, pcounts = _prepare(
        latents, actions, policy_indices, W1, b1, W2, b2, W3, b3
    )
    res = run_bass_kernel_spmd(nc, in_maps, list(range(NCORES)), trace=TRACE)
    LAST_RESULT = res
    b3 = np.asarray(b3, dtype=np.float32)
    return _scatter_out(
        res.results, order, counts, pcounts,
        np.asarray(latents).shape[0], None, b3,
    )


# revision 28
# speedup vs baseline: 1.0482x; 1.0482x over previous
"""Inner-policy-sharded Trainium2 kernel for DecoupledDynamicsModel (MoE),
fp8-DoubleRow edition.

Model: B=8192 rows; each row selects one of P=8 outer policies via
policy_indices; the selected policy runs 8 inner MLPs (72 -> 512 -> 512 -> 64)
on (latent chunk, action); the 8 inner outputs concatenate to 512 dims.

Sharding: by INNER policy (as the previous fp32r kernel): core i computes
inner MLP i for every row, rows sorted by outer policy so weights are
stationary per contiguous group.

Compute strategy (vs the 24 cyc/token fp32r baseline -> 16 cyc/token):
 - Layer 1 (K=72): fp8 e4m3 DoubleRow matmul with an error-compensated
   3-term expansion packed into one K=216 contraction:
       W1q.x_hi + W1q.x_lo + r1.x_hi   (x = x_hi + x_lo exact fp8 pair,
                                        W1 = W1q + r1 fp8 pair)
   packed [108, 2, *] -> ONE DR matmul per m-chunk = 0.5 cyc/row -> 2T.
 - Layer 2 (K=512): same 3-term scheme, 6 DR matmuls per m-chunk -> 12T.
   The moving pair (h_hi, h_lo) is produced on-chip: ACT evicts
   h = s_h*relu(psum*a + b) to bf16, a software-DGE SBUF->SBUF casting DMA
   makes h_hi = fp8(h), and DVE/Pool subtract h - h_hi -> h_lo (fp8).
   The hi+lo pair reconstructs h exactly up to fp8(h_lo) rounding, so the
   dropped r2.h_lo cross term is the only systematic error (~0.4%).
 - Layer 3: flipped orientation bf16: stationary = h2 [128h, 128tok]
   chunks, moving = W3 [128, 64] -> out [tok, 64] in PSUM, 2T.
Measured end-to-end emulation rel_err ~3.6e-3 (gate 2e-2).

All scales are powers of two folded into ACT scale/bias; the final
1/(s_h*sw2) descale and +b3 happen on the host after the gather.
"""

import sys

sys.path.insert(0, "/opt/trn_rl_repo")

import numpy as np
import ml_dtypes

import concourse.bass as bass
from concourse import bacc
import concourse.mybir as mybir
import concourse.tile as tile
from concourse.bass_utils import run_bass_kernel_spmd

P = 8          # outer policies == n_cores == inner MLPs per policy
Z = 64         # per-policy latent dim
D = P * Z      # 512
A = 8          # action dim
IN = Z + A     # 72
H = 512        # hidden dim
NCORES = 8

SX = 16.0      # x fp8 scale
SW1 = 2.0      # W1 fp8 scale (SX*SW1 == SH so layer-1 eviction has scale 1)
SH = 32.0      # h fp8/bf16 scale
SW2 = 2048.0   # W2 fp8 scale
A1 = SH / (SX * SW1)     # ACT scale for layer-1 eviction (2^-8)
SIG2 = SH * SW2          # scale of layer-2/3 psum; undone on host

W8C = 5120     # fp8 weight tile columns per group: w1 1024 | wq2a/b | r2a/b
W16C = 256     # bf16 tile: W3 [128, 4*64]

F8 = mybir.dt.float8e4
BF = mybir.dt.bfloat16
F32 = mybir.dt.float32
RELU = mybir.ActivationFunctionType.Relu
COPY = mybir.ActivationFunctionType.Copy
DR = mybir.MatmulPerfMode.DoubleRow
ADD = mybir.AluOpType.add
MAX = mybir.AluOpType.max
SUB = mybir.AluOpType.subtract
NF8 = ml_dtypes.float8_e4m3
NBF = ml_dtypes.bfloat16

TRACE = False
REPEAT = 1
LAST_RESULT = None


def _q8(v):
    """Round to fp8 e4m3 (framework dtype), back to f32."""
    return v.astype(NF8).astype(np.float32)


def _group_tiles(counts):
    """Token tiles of <=512 inside each outer-policy group (counts padded
    to multiples of 4). Group 0 leads with small tiles so the software
    pipeline (depth 4) fills while DMAs are still streaming."""
    tiles = []
    off = 0
    for g, n in enumerate(counts):
        r = n
        lead = [128, 128, 256] if g == 0 else []
        for t in lead:
            if r >= t + 128:
                tiles.append((g, off, t))
                off += t
                r -= t
        while r > 0:
            t = 512 if r >= 512 else r
            tiles.append((g, off, t))
            off += t
            r -= t
    return tiles


def _subtile_plan(tiles):
    """Global subtile index space for the token-major y output: per tile a
    (j0, widths) entry; widths are <=128 chunks of the tile."""
    plan = []
    j = 0
    for (_g, _t0, tw) in tiles:
        widths = []
        r = tw
        while r > 0:
            w = 128 if r >= 128 else r
            widths.append(w)
            r -= w
        plan.append((j, widths))
        j += len(widths)
    return plan, j


def _build_program(counts, B, repeat=1):
    tiles = _group_tiles(counts)
    plan, nsub = _subtile_plan(tiles)
    NT = len(tiles)

    nc = bacc.Bacc()
    xd = nc.declare_dram_parameter("x8", [108, 2, B], F8, isOutput=False)
    w8d = nc.declare_dram_parameter("w8", [P, 128, W8C], F8, isOutput=False)
    w16d = nc.declare_dram_parameter("w16", [128, P * 4, 64], BF, isOutput=False)
    w32d = nc.declare_dram_parameter("w32", [128, P * 8], F32, isOutput=False)
    yd = nc.declare_dram_parameter("y", [128, nsub, 64], F32, isOutput=True)

    with tile.TileContext(nc) as tc:
        with (
            tc.tile_pool(name="xs", bufs=1) as xpool,
            tc.tile_pool(name="w", bufs=P) as wpool,
            tc.tile_pool(name="b", bufs=1) as bpool,
            tc.tile_pool(name="h", bufs=5) as hpool,
            tc.tile_pool(name="ys", bufs=2) as ypool,
            tc.tile_pool(name="ps1", bufs=7, space="PSUM") as pp1,
            tc.tile_pool(name="ps3", bufs=1, space="PSUM") as pp3,
        ):
            for _rep in range(repeat):
                xt = xpool.tile([108, 2, B], F8, tag="x")
                bia = bpool.tile([128, P * 8], F32, tag="bias")
                w3a = bpool.tile([128, P * 4, 64], BF, tag="w3a")
                # one packed fp8 weight tile per group; regions are 3D views
                wg8 = []
                for _g in range(P):
                    wg8.append(
                        wpool.tile([128, 10, 512], F8, tag="wg8", name=f"wg8_{_g}")
                    )

                def w1s(g):
                    return wg8[g][0:108, 0:2, :]

                def wq2(g, half):
                    return wg8[g][:, 2 + 2 * half : 4 + 2 * half, :]

                def r2(g, half):
                    return wg8[g][:, 6 + 2 * half : 8 + 2 * half, :]

                # ---- just-in-time DMA chunk schedule ----
                # weight chunks sized <=1us so pair-cast transfers never
                # queue behind bulk weights on the shared DMA device
                first_slot = {}
                for k, (g, _t0, _tw) in enumerate(tiles):
                    first_slot.setdefault(g, k)

                def wchunk(g, d0, d1):
                    return lambda: nc.sync.dma_start(
                        wg8[g][:, d0:d1, :], w8d[g, :, d0 * 512 : d1 * 512]
                    )

                dma_sched = {}

                def sched(slot, thunk):
                    dma_sched.setdefault(max(slot, 0), []).append(thunk)

                for g in range(P):
                    fs = first_slot[g]
                    if g > 0:
                        sched(fs - 4, wchunk(g, 0, 2))
                    sched(fs - 2, wchunk(g, 2, 6))
                    sched(fs - 1, wchunk(g, 6, 10))
                sched(3, lambda: nc.sync.dma_start(
                    w3a[:, 0:16, :], w16d[:, 0:16, :]))
                sched(5, lambda: nc.sync.dma_start(
                    w3a[:, 16:32, :], w16d[:, 16:32, :]))
                # x spans: cover through tile s+2 at slot s
                xcov = [0]

                def x_to(tok):
                    if tok > xcov[0]:
                        nc.sync.dma_start(
                            xt[:, :, xcov[0] : tok], xd[:, :, xcov[0] : tok]
                        )
                        xcov[0] = tok

                # prologue DMAs
                nc.sync.dma_start(bia[:, :], w32d[:, :])
                nc.sync.dma_start(wg8[0][:, 0:2, :], w8d[0, :, 0:1024])
                x_to(tiles[2][1] + tiles[2][2] if NT > 2 else B)

                # ---- software-pipelined tile loop ----
                # pair-batched h buffers: tiles 2p, 2p+1 share [128, 4, 1024]
                # packed contiguously (tile 2p+1 at column tiles[2p].tw)
                hbs, hhs, hls, h2s = {}, {}, {}, {}
                poff = {}
                for k, (_g, _t0, tw) in enumerate(tiles):
                    poff[k] = 0 if k % 2 == 0 else tiles[k - 1][2]

                # greedy engine placement: every psum eviction can run on
                # ACT (activation) or DVE (tensor_scalar); assign each op to
                # whichever projected engine load is lower. Subs split
                # DVE/Pool the same way.
                load = {"A": 0.0, "D": 0.0, "P": 0.0}

                def place_evict(dst, src, bias_ap, cols):
                    ca = cols * 0.833 + 200
                    cd = cols * 1.0417 + 195
                    if load["A"] + ca <= load["D"] + cd:
                        load["A"] += ca
                        nc.scalar.activation(dst, src, RELU, bias=bias_ap,
                                             scale=1.0)
                    else:
                        load["D"] += cd
                        nc.vector.tensor_scalar(dst, src, bias_ap, 0.0,
                                                ADD, MAX)

                def place_copy(dst, src, cols):
                    ca = cols * 0.833 + 200
                    cd = cols * 1.0417 + 195
                    if load["A"] + ca <= load["D"] + cd:
                        load["A"] += ca
                        nc.scalar.activation(dst, src, COPY)
                    else:
                        load["D"] += cd
                        nc.vector.tensor_copy(dst, src)

                def place_sub(h_lo, h_bf, h_hi, r0, r1, off, tw):
                    cols = (r1 - r0) * tw
                    cd = cols * 1.0417 + 195
                    cp = cols * 1.984 + 60
                    a = (h_lo[:, r0:r1, off : off + tw],
                         h_bf[:, r0:r1, off : off + tw],
                         h_hi[:, r0:r1, off : off + tw], SUB)
                    if load["D"] + cd <= load["P"] + cp:
                        load["D"] += cd
                        nc.vector.tensor_tensor(*a)
                    else:
                        load["P"] += cp
                        nc.gpsimd.tensor_tensor(*a)

                def stage1(k, ms):
                    g, t0, tw = tiles[k]
                    p = k // 2
                    if k % 2 == 0 and ms[0] == 0:
                        hbs[p] = hpool.tile([128, 4, 1024], BF, tag="hb", name=f"hb_{p}")
                        hhs[p] = hpool.tile([128, 4, 1024], F8, tag="hh", name=f"hh_{p}")
                        hls[p] = hpool.tile([128, 4, 1024], F8, tag="hl", name=f"hl_{p}")
                    h_bf = hbs[p]
                    off = poff[k]
                    for m in ms:
                        ps = pp1.tile([128, 512], F32, tag="p1")
                        nc.tensor.matmul(
                            ps[:, :tw],
                            w1s(g)[:, :, m * 128 : (m + 1) * 128],
                            xt[:, :, t0 : t0 + tw],
                            start=True,
                            stop=True,
                            perf_mode=DR,
                        )
                        place_evict(
                            h_bf[:, m, off : off + tw], ps[:, :tw],
                            bia[:, g * 8 + m : g * 8 + m + 1], tw,
                        )

                def cast_subs(p, klast):
                    # one width-exact casting DMA for the pair, then subs;
                    # kpair0 subs pinned to DVE (their h_lo mms come first)
                    h_bf, h_hi, h_lo = hbs[p], hhs[p], hls[p]
                    W = poff[klast] + tiles[klast][2]
                    nc.gpsimd.dma_start(h_hi[:, :, :W], h_bf[:, :, :W])
                    load["P"] += 994 + W / 8
                    for k in range(2 * p, klast + 1):
                        _g, _t0, tw = tiles[k]
                        off = poff[k]
                        load["D"] += 2 * tw * 1.0417 + 195
                        nc.vector.tensor_tensor(
                            h_lo[:, 0:2, off : off + tw],
                            h_bf[:, 0:2, off : off + tw],
                            h_hi[:, 0:2, off : off + tw], SUB,
                        )
                        place_sub(h_lo, h_bf, h_hi, 2, 3, off, tw)
                        place_sub(h_lo, h_bf, h_hi, 3, 4, off, tw)

                def stage2(k):
                    g, t0, tw = tiles[k]
                    p, half = k // 2, k % 2
                    h_hi, h_lo = hhs[p], hls[p]
                    o = poff[k]
                    h2 = hpool.tile([128, 4, 512], BF, tag="h2")
                    h2s[k] = h2
                    # all 16 h_hi mms first (the cast unblocks them), then
                    # the 8 h_lo mms - the subs get ~1.5us of extra slack
                    pss = []
                    for m in range(4):
                        msl = slice(m * 128, (m + 1) * 128)
                        ps = pp1.tile([128, 512], F32, tag="p1")
                        pss.append(ps)
                        for j, (w, mv) in enumerate([
                            (wq2(g, 0), h_hi[:, 0:2, o : o + tw]),
                            (wq2(g, 1), h_hi[:, 2:4, o : o + tw]),
                            (r2(g, 0), h_hi[:, 0:2, o : o + tw]),
                            (r2(g, 1), h_hi[:, 2:4, o : o + tw]),
                        ]):
                            nc.tensor.matmul(
                                pss[m][:, :tw], w[:, :, msl], mv,
                                start=(j == 0), stop=False,
                                perf_mode=DR,
                            )
                    for m in range(4):
                        msl = slice(m * 128, (m + 1) * 128)
                        nc.tensor.matmul(
                            pss[m][:, :tw], wq2(g, 0)[:, :, msl],
                            h_lo[:, 0:2, o : o + tw],
                            start=False, stop=False, perf_mode=DR,
                        )
                    for m in range(4):
                        msl = slice(m * 128, (m + 1) * 128)
                        nc.tensor.matmul(
                            pss[m][:, :tw], wq2(g, 1)[:, :, msl],
                            h_lo[:, 2:4, o : o + tw],
                            start=False, stop=True, perf_mode=DR,
                        )
                        place_evict(
                            h2[:, m, :tw], pss[m][:, :tw],
                            bia[:, g * 8 + 4 + m : g * 8 + 4 + m + 1], tw,
                        )
                    if half == 1 or k == NT - 1:
                        hbs.pop(p), hhs.pop(p), hls.pop(p)

                def stage3(k):
                    g, t0, tw = tiles[k]
                    j0, widths = plan[k]
                    h2 = h2s.pop(k)
                    ps3 = pp3.tile([128, 4, 64], F32, tag="p3")
                    for s, w in enumerate(widths):
                        for kc in range(4):
                            nc.tensor.matmul(
                                ps3[0:w, s, :],
                                h2[:, kc, s * 128 : s * 128 + w],
                                w3a[:, g * 4 + kc, :],
                                start=(kc == 0),
                                stop=(kc == 3),
                            )
                    y = ypool.tile([128, 4, 64], F32, tag="y")
                    ns = len(widths)
                    nfull = ns if widths[-1] == 128 else ns - 1
                    if nfull:
                        place_copy(y[:, 0:nfull, :], ps3[:, 0:nfull, :],
                                   nfull * 64)
                        nc.sync.dma_start(
                            yd[:, j0 : j0 + nfull, :], y[:, 0:nfull, :]
                        )
                    if nfull < ns:
                        wl = widths[-1]
                        place_copy(
                            y[0:wl, nfull : nfull + 1, :],
                            ps3[0:wl, nfull : nfull + 1, :], 64,
                        )
                        nc.sync.dma_start(
                            yd[0:wl, j0 + nfull : j0 + nfull + 1, :],
                            y[0:wl, nfull : nfull + 1, :],
                        )

                for s in range(NT + 6):
                    for thunk in dma_sched.get(s, []):
                        thunk()
                    if s % 2 == 0 and s + 3 < NT:
                        x_to(tiles[s + 3][1] + tiles[s + 3][2])
                    elif s + 3 >= NT and xcov[0] < B:
                        x_to(B)
                    if s < NT:
                        stage1(s, (0, 1, 2))
                    if 0 <= s - 4 < NT:
                        stage2(s - 4)
                    if 0 <= s - 5 < NT:
                        stage3(s - 5)
                    if s < NT:
                        stage1(s, (3,))
                        if s % 2 == 1 or s == NT - 1:
                            cast_subs(s // 2, s)

    nc.finalize()
    return nc


def _pack_inputs(latents, actions, order, counts, Bp, W1, b1, W2, b2, W3, b3):
    """Per-core inputs. Core i: fp8 pair-packed x and per-(g, inner-i)
    quantized weights."""
    lat_s = latents[order]
    act_s = actions[order]
    # sorted+padded x, zero in pad columns
    spans = []
    po = ro = 0
    for n, pn in zip(counts, [c for c in counts]):
        pass
    in_maps = []
    # padded group layout
    offs = []
    po = ro = 0
    for n in counts:
        offs.append((po, ro, n))
        pn = -(-n // 4) * 4
        po += pn
        ro += n

    for i in range(NCORES):
        xf = np.zeros((IN, Bp), dtype=np.float32)
        for po_, ro_, n in offs:
            xf[:Z, po_ : po_ + n] = lat_s[ro_ : ro_ + n, i * Z : (i + 1) * Z].T
            xf[Z:, po_ : po_ + n] = act_s[ro_ : ro_ + n].T
        xs = xf * SX
        x_hi = _q8(xs)
        x_lo = _q8(xs - x_hi)
        flat = np.zeros((216, Bp), dtype=np.float32)
        flat[0:72] = x_hi
        flat[72:144] = x_lo
        flat[144:216] = x_hi
        x8 = flat.reshape(2, 108, Bp).transpose(1, 0, 2).astype(NF8)

        w8 = np.zeros((P, 128, W8C), dtype=NF8)
        w16 = np.zeros((128, P * 4, 64), dtype=NBF)
        w32 = np.zeros((128, P * 8), dtype=np.float32)
        for g in range(P):
            w1sc = W1[g, i] * SW1                    # [72, 512]
            w1q = _q8(w1sc)
            r1 = _q8(w1sc - w1q)
            flatw = np.concatenate([w1q, w1q, r1], 0)  # [216, 512]
            # [108, 2, 512] with flat row f = j*108+p -> cols j*512+c
            w8[g, 0:108, 0:1024] = (
                flatw.reshape(2, 108, 512).transpose(1, 0, 2).reshape(108, 1024)
            ).astype(NF8)

            w2sc = W2[g, i] * SW2                    # [512, 512]
            w2q = _q8(w2sc)
            r2 = _q8(w2sc - w2q)
            for src, base in ((w2q, 1024), (r2, 3072)):
                kt = src.reshape(4, 128, 512)        # [ktile, p, col]
                w8[g, :, base : base + 1024] = (
                    kt[0:2].transpose(1, 0, 2).reshape(128, 1024).astype(NF8)
                )
                w8[g, :, base + 1024 : base + 2048] = (
                    kt[2:4].transpose(1, 0, 2).reshape(128, 1024).astype(NF8)
                )

            w16[:, g * 4 : g * 4 + 4, :] = (
                W3[g, i].reshape(4, 128, Z).transpose(1, 0, 2)
            ).astype(NBF)
            w32[:, g * 8 : g * 8 + 4] = (SH * b1[g, i]).reshape(4, 128).T
            w32[:, g * 8 + 4 : g * 8 + 8] = (SIG2 * b2[g, i]).reshape(4, 128).T

        in_maps.append({"x8": x8, "w8": w8, "w16": w16, "w32": w32})
    return in_maps


def _prepare(latents, actions, policy_indices, W1, b1, W2, b2, W3, b3):
    latents = np.asarray(latents, dtype=np.float32)
    actions = np.asarray(actions, dtype=np.float32)
    idx = np.asarray(policy_indices).astype(np.int64)
    W1 = np.ascontiguousarray(np.asarray(W1, dtype=np.float32))
    W2 = np.ascontiguousarray(np.asarray(W2, dtype=np.float32))
    W3 = np.ascontiguousarray(np.asarray(W3, dtype=np.float32))
    b1 = np.asarray(b1, dtype=np.float32)
    b2 = np.asarray(b2, dtype=np.float32)
    b3 = np.asarray(b3, dtype=np.float32)

    order = np.argsort(idx, kind="stable")
    counts = np.bincount(idx, minlength=P).tolist()
    pcounts = [-(-n // 4) * 4 for n in counts]
    Bp = sum(pcounts)

    in_maps = _pack_inputs(
        latents, actions, order, counts, Bp, W1, b1, W2, b2, W3, b3
    )
    nc = _build_program(pcounts, Bp, repeat=REPEAT)
    return nc, in_maps, order, counts, pcounts


def _scatter_out(results, order, counts, pcounts, B, idx_sorted_groups, b3):
    tiles = _group_tiles(pcounts)
    plan, nsub = _subtile_plan(tiles)
    Bp = sum(pcounts)
    keep = np.zeros(Bp, dtype=bool)
    po = 0
    for n, pn in zip(counts, pcounts):
        keep[po : po + n] = True
        po += pn
    # group id per padded row (for b3)
    grow = np.zeros(Bp, dtype=np.int64)
    po = 0
    for g, pn in enumerate(pcounts):
        grow[po : po + pn] = g
        po += pn

    out = np.empty((B, D), dtype=np.float32)
    for i in range(NCORES):
        yO = results[i]["y"]                      # [128, nsub, 64]
        ys = np.empty((Bp, Z), dtype=np.float32)
        for (g, t0, tw), (j0, widths) in zip(tiles, plan):
            o = t0
            for s, w in enumerate(widths):
                ys[o : o + w] = yO[0:w, j0 + s, :]
                o += w
        ys = ys / SIG2 + b3[grow, i]
        out[order, i * Z : (i + 1) * Z] = ys[keep]
    return out


def kernel(latents, actions, policy_indices, W1, b1, W2, b2, W3, b3):
    global LAST_RESULT
    nc, in_maps, order, counts, pcounts = _prepare(
        latents, actions, policy_indices, W1, b1, W2, b2, W3, b3
    )
    res = run_bass_kernel_spmd(nc, in_maps, list(range(NCORES)), trace=TRACE)
    LAST_RESULT = res
    b3 = np.asarray(b3, dtype=np.float32)
    return _scatter_out(
        res.results, order, counts, pcounts,
        np.asarray(latents).shape[0], None, b3,
    )
